# revision 1
# baseline (speedup 1.0000x reference)
"""Trainium2 Bass/Tile kernel: symmetric contrastive loss (CLIP-style).

Distribution: data-parallel over B across 8 NeuronCores.  Each core MLPs +
l2-normalizes its 2048-row shard of both branches, AllGathers the normalized
num-projections (bf16, 512KB/rank), computes its row-block of the 16384^2
logit matrix tile-by-tile (never materialized), and reduces:

  * rows  (i2n): ACT Exp with fused accum_out -> per-row sum(exp) locally
  * cols  (n2i): ones-matmul partition sums accumulated in PSUM, then one
    AllReduce-add of [colsum(16384) | sum(lse_rows) - sum(diag) | sum(diag)]

Logits are bounded (|cos|/temp <= 10) so logsumexp needs no max shift; plain
fp32 exp-sums are exact enough.  Temperature is folded into the projections
via scale 1/sqrt(temp) so no runtime scalar is needed inside Exp; the l2
normalization itself is exp(-0.5*ln(|z|^2) - 0.5*log_temp) on ACT (Rsqrt on
ACT is banned for accuracy).
"""

import numpy as np

N_CORES = 8
B = 16384
D_IMG = 2048
D_NUM = 256
P = 128

_NC_CACHE = {}


def build(b_total=B, d_img=D_IMG, d_num=D_NUM, n_cores=N_CORES):
    """Build + compile the Bass module. Returns the compiled Bacc object."""
    key = (b_total, d_img, d_num, n_cores)
    if key in _NC_CACHE:
        return _NC_CACHE[key]

    import concourse.bacc as bacc
    import concourse.bass as bass
    import concourse.mybir as mybir
    import concourse.tile as tile

    dt = mybir.dt
    AF = mybir.ActivationFunctionType
    Alu = mybir.AluOpType
    AX = mybir.AxisListType
    f32 = dt.float32
    bf16 = dt.bfloat16

    BL = b_total // n_cores          # local rows per core
    assert BL % 512 == 0 and b_total % 1024 == 0
    NRT = BL // 512                  # 512-wide row tiles (MLP / transpose)
    NRC = BL // 128                  # 128-row chunks (main pass)
    KI = d_img // 128                # contraction tiles, img MLP1
    KN = d_num // 128
    CW = 1024                        # main-pass column supertile width
    NCT = b_total // CW
    NH = CW // 512
    ARW = b_total + 64               # AllReduce payload width

    nc = bacc.Bacc("TRN2", target_bir_lowering=False, debug=False,
                   num_devices=n_cores)

    img = nc.dram_tensor("img_feat", [BL, d_img], f32, kind="ExternalInput").ap()
    num = nc.dram_tensor("num_feat", [BL, d_num], f32, kind="ExternalInput").ap()
    Wi1 = nc.dram_tensor("Wi1", [d_img, P], f32, kind="ExternalInput").ap()
    bi1 = nc.dram_tensor("bi1", [P, 1], f32, kind="ExternalInput").ap()
    Wi2 = nc.dram_tensor("Wi2", [P, P], f32, kind="ExternalInput").ap()
    bi2 = nc.dram_tensor("bi2", [P, 1], f32, kind="ExternalInput").ap()
    Wn1 = nc.dram_tensor("Wn1", [d_num, P], f32, kind="ExternalInput").ap()
    bn1 = nc.dram_tensor("bn1", [P, 1], f32, kind="ExternalInput").ap()
    Wn2 = nc.dram_tensor("Wn2", [P, P], f32, kind="ExternalInput").ap()
    bn2 = nc.dram_tensor("bn2", [P, 1], f32, kind="ExternalInput").ap()
    ltm = nc.dram_tensor("log_temp", [1, 1], f32, kind="ExternalInput").ap()
    loss = nc.dram_tensor("loss", [1, 1], f32, kind="ExternalOutput").ap()

    rg = [list(range(n_cores))]

    with tile.TileContext(nc) as tc:
        with (
            tc.tile_pool(name="sb", bufs=1) as sb,
            tc.tile_pool(name="stream", bufs=3) as st,
            tc.tile_pool(name="vstage", bufs=2) as vs,
            tc.tile_pool(name="xtp", bufs=2) as xtp,
            tc.tile_pool(name="xsp", bufs=2) as xsp,
            tc.tile_pool(name="dram", bufs=1, space="DRAM") as dram,
        ):
            # ---------------- constants ----------------
            ones_kb = sb.tile([P, 1], bf16)
            nc.vector.memset(ones_kb[:], 1.0)
            ones_kf = sb.tile([P, 1], f32)
            nc.vector.memset(ones_kf[:], 1.0)
            ones_1f = sb.tile([1, P], f32)
            nc.vector.memset(ones_1f[:], 1.0)
            zpad = sb.tile([1, 64], f32)
            nc.vector.memset(zpad[:], 0.0)
            # identity (bf16) for PE-mode transposes: (free_idx - part_idx)==0
            idn_i = sb.tile([P, P], dt.int32)
            nc.gpsimd.iota(idn_i[:], pattern=[[1, P]], base=0,
                           channel_multiplier=-1)
            idn = sb.tile([P, P], bf16)
            nc.vector.tensor_scalar(idn[:], idn_i[:], 0, None,
                                    op0=Alu.is_equal)
            idn_f = sb.tile([P, P], f32)
            nc.vector.tensor_scalar(idn_f[:], idn_i[:], 0, None,
                                    op0=Alu.is_equal)

            # num input first -- it gates the whole AllGather chain.
            xs_n = sb.tile([P, NRC, d_num], f32)
            nc.sync.dma_start(xs_n[:], num.rearrange("(g p) e -> p g e", p=P))

            # num-branch weights next: loaded via HWDGE (sync) as fp32 + DVE
            # cast so they never queue behind the big img SWDGE cast-DMAs.
            wn1_f = sb.tile([P, KN * P], f32)
            nc.sync.dma_start(wn1_f.rearrange("p (k m) -> p k m", k=KN),
                              Wn1.rearrange("(k p) m -> p k m", p=P))
            wn1_sb = sb.tile([P, KN * P], bf16)
            nc.vector.tensor_copy(wn1_sb[:], wn1_f[:])
            wn2_f = sb.tile([P, P], f32)
            nc.sync.dma_start(wn2_f[:], Wn2)
            wn2_sb = sb.tile([P, P], bf16)
            nc.vector.tensor_copy(wn2_sb[:], wn2_f[:])
            bn1_sb = sb.tile([P, 1], f32)
            nc.sync.dma_start(bn1_sb[:], bn1)
            bn2_sb = sb.tile([P, 1], f32)
            nc.sync.dma_start(bn2_sb[:], bn2)
            lt_sb = sb.tile([1, 1], f32)
            nc.sync.dma_start(lt_sb[:], ltm)
            nhlt = sb.tile([1, 1], f32)        # -0.5 * log_temp
            nc.vector.tensor_scalar_mul(nhlt[:], lt_sb[:], -0.5)

            # ---------------- DRAM scratch ----------------
            # AllGather split in two halves so the main pass can start after
            # the first half lands; AllReduce split so half overlaps compute.
            BH = BL // 2
            ag_in_a = dram.tile([P, BH], bf16)
            ag_in_b = dram.tile([P, BH], bf16)
            ag_out_a = dram.tile([n_cores * P, BH], bf16, addr_space="Shared")
            ag_out_b = dram.tile([n_cores * P, BH], bf16, addr_space="Shared")
            ARH = b_total // 2
            ar_in = dram.tile([1, ARW], f32)
            ar_out_a = dram.tile([1, ARH], f32, addr_space="Shared")
            ar_out_b = dram.tile([1, ARW - ARH], f32, addr_space="Shared")

            # ---------------- persistent SBUF ----------------
            xnT = sb.tile([P, KN * BL], bf16)   # num input, transposed
            h1n = sb.tile([P, BL], bf16)
            h1i = sb.tile([P, BL], bf16)
            zn = sb.tile([P, BL], bf16)
            zi = sb.tile([P, BL], bf16)
            ntl = sb.tile([P, BL], bf16)        # normalized num proj (local)
            itl = sb.tile([P, BL], bf16)        # normalized img proj (local)
            npf = sb.tile([P, b_total], bf16)   # gathered num proj (all cores)
            rowacc = sb.tile([P, NRC * NCT], f32)
            dsum = sb.tile([1, 1], f32)         # running sum of diag
            nc.vector.memset(dsum[:], 0.0)

            def mlp2_norm(pp, h1, w2, b2, z, outp):
                """z = w2.T@h1 + b2 (transposed layout); outp = z * inv, with
                inv[i] = exp(-0.5*ln(|z_i|^2) - 0.5*log_temp).  Per-row-tile
                so outp slices become ready incrementally (prologue latency
                matters more than the extra ACT table switches, which land in
                otherwise-idle ACT time)."""
                for rt in range(NRT):
                    sl = slice(rt * 512, (rt + 1) * 512)
                    pz = pp.tile([P, 512], f32, tag="zb", name="pz")
                    nc.tensor.matmul(pz[:], w2[:], h1[:, sl])
                    nc.scalar.activation(z[:, sl], pz[:], AF.Identity, bias=b2[:])
                    sq = st.tile([P, 512], bf16, tag="sq", name="sq")
                    nc.scalar.activation(sq[:], pz[:], AF.Square, bias=b2[:])
                    pv = pp.tile([P, 512], f32, tag="v", name="pv")
                    nc.tensor.matmul(pv[:1, :], ones_kb[:], sq[:])
                    lnv = vs.tile([1, 512], f32, tag="lnv", name="lnv")
                    nc.scalar.activation(lnv[:], pv[:1, :], AF.Ln)
                    inv = vs.tile([1, 512], f32, tag="inv", name="inv")
                    nc.scalar.activation(inv[:], lnv[:], AF.Exp,
                                         bias=nhlt[:], scale=-0.5)
                    pb = pp.tile([P, 512], f32, tag="zb", name="pb")
                    nc.tensor.matmul(pb[:], ones_1f[:], inv[:])
                    nc.vector.tensor_mul(outp[:, sl], z[:, sl], pb[:])

            # ---------------- num branch + AllGather ----------------
            with tc.tile_pool(name="pp1", bufs=2, space="PSUM") as pp:
                # f32 PE transposes + cast copy from the early xs_n load
                for dk in range(KN):
                    for gb in range(NRC // 4):
                        pt = pp.tile([P, 512], f32, tag="pt", name="ptn")
                        for q in range(4):
                            nc.tensor.transpose(
                                pt[:, q * P:(q + 1) * P],
                                xs_n[:, gb * 4 + q, dk * P:(dk + 1) * P],
                                idn_f[:])
                        nc.vector.tensor_copy(
                            xnT[:, dk * BL + gb * 512: dk * BL + gb * 512 + 512],
                            pt[:])
                for rt in range(NRT):
                    sl = slice(rt * 512, (rt + 1) * 512)
                    ph = pp.tile([P, 512], f32, tag="h", name="ph")
                    for k in range(KN):
                        nc.tensor.matmul(
                            ph[:], wn1_sb[:, k * P:(k + 1) * P],
                            xnT[:, k * BL + rt * 512: k * BL + rt * 512 + 512],
                            start=(k == 0), stop=(k == KN - 1))
                    nc.scalar.activation(h1n[:, sl], ph[:], AF.Relu, bias=bn1_sb[:])
                mlp2_norm(pp, h1n, wn2_sb, bn2_sb, zn, ntl)

            npf_v = npf.rearrange("p (r c) -> p r c", c=BL)
            nc.sync.dma_start(ag_in_a[:], ntl[:, 0:BH])
            nc.gpsimd.collective_compute(
                "AllGather", Alu.bypass, replica_groups=rg,
                ins=[ag_in_a.opt()], outs=[ag_out_a.opt()])
            nc.sync.dma_start(ag_in_b[:], ntl[:, BH:BL])
            nc.gpsimd.collective_compute(
                "AllGather", Alu.bypass, replica_groups=rg,
                ins=[ag_in_b.opt()], outs=[ag_out_b.opt()])
            nc.sync.dma_start(npf_v[:, :, 0:BH],
                              ag_out_a.rearrange("(r p) n -> p r n", p=P))
            nc.sync.dma_start(npf_v[:, :, BH:BL],
                              ag_out_b.rearrange("(r p) n -> p r n", p=P))

            # img-branch weights (after AG trigger; not on its critical path)
            wi1_sb = sb.tile([P, KI * P], bf16)
            nc.gpsimd.dma_start(wi1_sb.rearrange("p (k m) -> p k m", k=KI),
                                Wi1.rearrange("(k p) m -> p k m", p=P))
            wi2_sb = sb.tile([P, P], bf16)
            nc.gpsimd.dma_start(wi2_sb[:], Wi2)
            bi1_sb = sb.tile([P, 1], f32)
            nc.sync.dma_start(bi1_sb[:], bi1)
            bi2_sb = sb.tile([P, 1], f32)
            nc.sync.dma_start(bi2_sb[:], bi2)

            # ---------------- img branch ----------------
            with tc.tile_pool(name="pp2", bufs=2, space="PSUM") as pp:
                for rb in range(NRT):
                    rsl = slice(rb * 512, (rb + 1) * 512)
                    xs = xsp.tile([P, 4, d_img], bf16, tag="xsi", name="xsi")
                    nc.gpsimd.dma_start(
                        xs[:], img[rsl, :].rearrange("(q p) e -> p q e", p=P))
                    xtb = xtp.tile([P, KI * 512], bf16, tag="xt", name="xtb")
                    for dk in range(KI):
                        pt = pp.tile([P, 512], bf16, tag="pt", name="pt")
                        for q in range(4):
                            nc.tensor.transpose(
                                pt[:, q * P:(q + 1) * P],
                                xs[:, q, dk * P:(dk + 1) * P], idn[:])
                        nc.vector.tensor_copy(
                            xtb[:, dk * 512:(dk + 1) * 512], pt[:])
                    ph = pp.tile([P, 512], f32, tag="h", name="phi")
                    for k in range(KI):
                        nc.tensor.matmul(
                            ph[:], wi1_sb[:, k * P:(k + 1) * P],
                            xtb[:, k * 512:(k + 1) * 512],
                            start=(k == 0), stop=(k == KI - 1))
                    nc.scalar.activation(h1i[:, rsl], ph[:], AF.Relu,
                                         bias=bi1_sb[:])
                mlp2_norm(pp, h1i, wi2_sb, bi2_sb, zi, itl)
                # diagonal: l_ii = sum_p itl[p,i] * ntl[p,i]; accumulate sum
                for rt in range(NRT):
                    sl = slice(rt * 512, (rt + 1) * 512)
                    prod = st.tile([P, 512], bf16, tag="sq", name="prod")
                    nc.vector.tensor_mul(prod[:], itl[:, sl], ntl[:, sl])
                    pd = pp.tile([P, 512], f32, tag="v", name="pd")
                    nc.tensor.matmul(pd[:1, :], ones_kb[:], prod[:])
                    dred = vs.tile([1, 1], f32, tag="dred", name="dred")
                    nc.vector.reduce_sum(dred[:], pd[:1, :], axis=AX.X)
                    nc.vector.tensor_add(dsum[:], dsum[:], dred[:])

            # ---------------- main pass ----------------
            # ct processing order: supertiles fully covered by AG half a
            # first, so the main pass starts before AG half b completes.
            ct_a = [ct for ct in range(NCT)
                    if (ct * CW) % BL + CW <= BH]
            ct_order = ct_a + [ct for ct in range(NCT) if ct not in ct_a]
            NHALF = NCT // 2
            with (
                tc.tile_pool(name="pl", bufs=3, space="PSUM") as plp,
                tc.tile_pool(name="pc", bufs=1, space="PSUM") as pcp,
            ):
                for pos, ct in enumerate(ct_order):
                    pcol = pcp.tile([P, CW], f32, tag="pc", name="pcol")
                    for rc in range(NRC):
                        plog = plp.tile([P, CW], f32, tag="pl", name="plog")
                        for h in range(NH):
                            nc.tensor.matmul(
                                plog[:, h * 512:(h + 1) * 512],
                                itl[:, rc * P:(rc + 1) * P],
                                npf[:, ct * CW + h * 512: ct * CW + (h + 1) * 512])
                        e = st.tile([P, CW], bf16, tag="e", name="e", bufs=4)
                        slot = rc * NCT + ct
                        nc.scalar.activation(e[:], plog[:], AF.Exp,
                                             accum_out=rowacc[:, slot:slot + 1])
                        for h in range(NH):
                            nc.tensor.matmul(
                                pcol[:1, h * 512:(h + 1) * 512],
                                ones_kb[:], e[:, h * 512:(h + 1) * 512],
                                start=(rc == 0), stop=(rc == NRC - 1))
                    cst = vs.tile([1, CW], f32, tag="cst", name="cst")
                    nc.vector.tensor_copy(cst[:], pcol[:1, :])
                    nc.sync.dma_start(ar_in[:1, pos * CW:(pos + 1) * CW], cst[:])
                    if pos == NHALF - 1:
                        # first half of colsums complete -> overlap AllReduce
                        nc.gpsimd.collective_compute(
                            "AllReduce", Alu.add, replica_groups=rg,
                            ins=[ar_in[:1, 0:ARH].opt()], outs=[ar_out_a.opt()])

                # ---- row direction partials ----
                rowsum = sb.tile([P, NRC], f32)
                nc.vector.reduce_sum(
                    rowsum[:],
                    rowacc.rearrange("p (rc ct) -> p rc ct", ct=NCT), axis=AX.X)
                lse_r = sb.tile([P, NRC], f32)
                nc.scalar.activation(lse_r[:], rowsum[:], AF.Ln)
                lsum = sb.tile([P, 1], f32)
                nc.vector.reduce_sum(lsum[:], lse_r[:], axis=AX.X)
                pR = pcp.tile([P, CW], f32, tag="pc", name="pR")
                nc.tensor.matmul(pR[:1, :1], ones_kf[:], lsum[:])
                rpart = sb.tile([1, 1], f32)
                nc.vector.tensor_sub(rpart[:], pR[:1, :1], dsum[:])
                nc.sync.dma_start(ar_in[:1, b_total:b_total + 1], rpart[:])
                nc.sync.dma_start(ar_in[:1, b_total + 1:b_total + 2], dsum[:])
                nc.sync.dma_start(ar_in[:1, b_total + 2:ARW], zpad[:1, :62])

                # ---- AllReduce (second half + scalars) ----
                nc.gpsimd.collective_compute(
                    "AllReduce", Alu.add, replica_groups=rg,
                    ins=[ar_in[:1, ARH:ARW].opt()], outs=[ar_out_b.opt()])

                # ---- final ----
                # sum_j log(colsum_j) is order-independent, so the permuted
                # (pos-ordered) colsum layout needs no unscrambling.
                HB2 = b_total - ARH
                csb = sb.tile([P, b_total // P], f32)
                nc.sync.dma_start(
                    csb[:, :ARH // P],
                    ar_out_a.rearrange("o (a b) -> (o a) b", a=P))
                nc.sync.dma_start(
                    csb[:, ARH // P:],
                    ar_out_b[:1, :HB2].rearrange("o (a b) -> (o a) b", a=P))
                sc2 = sb.tile([1, 2], f32)
                nc.sync.dma_start(sc2[:], ar_out_b[:1, HB2:HB2 + 2])
                lse_c = sb.tile([P, b_total // P], f32)
                nc.scalar.activation(lse_c[:], csb[:], AF.Ln)
                csum_p = sb.tile([P, 1], f32)
                nc.vector.reduce_sum(csum_p[:], lse_c[:], axis=AX.X)
                pC = pcp.tile([P, CW], f32, tag="pc", name="pC")
                nc.tensor.matmul(pC[:1, :1], ones_kf[:], csum_p[:])
                t1 = sb.tile([1, 1], f32)
                nc.vector.tensor_add(t1[:], pC[:1, :1], sc2[:1, 0:1])
                t2 = sb.tile([1, 1], f32)
                nc.vector.tensor_sub(t2[:], t1[:], sc2[:1, 1:2])
                lsb = sb.tile([1, 1], f32)
                nc.vector.tensor_scalar_mul(lsb[:], t2[:], 1.0 / (2.0 * b_total))
                nc.sync.dma_start(loss, lsb[:])

    nc.compile()
    _NC_CACHE[key] = nc
    return nc


def shard_inputs(inputs, b_total=B, n_cores=N_CORES):
    BL = b_total // n_cores
    img = np.ascontiguousarray(np.asarray(inputs["img_feat"], dtype=np.float32))
    num = np.ascontiguousarray(np.asarray(inputs["num_feat"], dtype=np.float32))

    def mat(name):
        return np.ascontiguousarray(np.asarray(inputs[name], dtype=np.float32))

    def col(name):
        return np.ascontiguousarray(
            np.asarray(inputs[name], dtype=np.float32).reshape(P, 1))

    lt = np.asarray(inputs["log_temp"], dtype=np.float32).reshape(1, 1)
    shared = {
        "Wi1": mat("Wi1"), "Wi2": mat("Wi2"),
        "Wn1": mat("Wn1"), "Wn2": mat("Wn2"),
        "bi1": col("bi1"), "bi2": col("bi2"),
        "bn1": col("bn1"), "bn2": col("bn2"),
        "log_temp": np.ascontiguousarray(lt),
    }
    maps = []
    for c in range(n_cores):
        m = dict(shared)
        m["img_feat"] = np.ascontiguousarray(img[c * BL:(c + 1) * BL])
        m["num_feat"] = np.ascontiguousarray(num[c * BL:(c + 1) * BL])
        maps.append(m)
    return maps


def run(inputs, trace=False, **kw):
    """Run on hardware; returns (loss_scalar, BassKernelResults)."""
    from concourse.bass_utils import run_bass_kernel_spmd
    nc = build()
    res = run_bass_kernel_spmd(nc, shard_inputs(inputs),
                               core_ids=list(range(N_CORES)), trace=trace, **kw)
    val = np.asarray(res.results[0]["loss"], dtype=np.float32).reshape(())
    return val, res


def kernel(**inputs):
    val, _ = run(inputs)
    return val



# revision 13
# speedup vs baseline: 1.0349x; 1.0349x over previous
"""Trainium2 Bass/Tile kernel: symmetric contrastive loss (CLIP-style).

Distribution: data-parallel over B across 8 NeuronCores.  Each core MLPs +
l2-normalizes its 2048-row shard of both branches, AllGathers the normalized
num-projections (bf16, 512KB/rank), computes its row-block of the 16384^2
logit matrix tile-by-tile (never materialized), and reduces rows (i2n) and
columns (n2i partials, AllReduce-add at the end).

Engine split in the main pass (the exp of 33.5M logits/core is the wall):
row-chunks are divided between
  * ACT rows: ACT Exp with fused accum_out -> per-row sum; PE ones-matmul
    accumulates column sums in PSUM.
  * DVE rows: Schraudolph exp on the Vector engine -- tensor_scalar computes
    round(x*128/ln2 + magic) into int16 whose bit pattern IS bf16 exp(x)
    (max 4.2% elementwise noise, mean-calibrated to <0.2%; irrelevant after
    16K-element sums), then one fused custom-DVE op adds e into a column
    accumulator and folds a row-sum (telescoped) in the same pass.

Logits are bounded (|cos|/temp <= 10) so no max shift is needed.  The l2
normalization is exp(-0.5*ln(|z|^2) - 0.5*log_temp) on ACT, batched so the
whole kernel needs ~3 activation-table loads; Identity/Square work is done
on DVE instead of ACT.
"""

import numpy as np

N_CORES = 8
B = 16384
D_IMG = 2048
D_NUM = 256
P = 128

# Schraudolph constants (bf16-target): int16 bits = x*128/ln2 + SCH_B.
# SCH_B calibrated midway between the truncate (16249.15) and round-to-
# nearest (16248.65) conventions so either hardware behavior keeps the
# mean multiplicative bias of exp under 0.2%.
SCH_A = 128.0 / float(np.log(2.0))
SCH_B = 16248.90

_NC_CACHE = {}
_DVE_OPS = {}


def _register_dve_ops():
    """Register the two fused DVE ops used by the D-path (runtime append to
    dve_ops.OPS; sha computed from lower() so the pin always matches)."""
    if _DVE_OPS:
        return _DVE_OPS
    from concourse.dve_ops import DveOp, OPS, CUSTOM_DVE_SPECS, _SUB_OPCODE_FOR_NAME
    from concourse.dve_spec import Spec, Src0, Src1, Zero, lower
    from concourse.dve_spec import _has_src1 as has_src1
    from concourse.dve_uop import DveOpSpec
    from operator import add

    def _ref_sum(body_fn):
        def _r(in0, in1, c0, c1, c2):
            b = body_fn(in0, in1, c0, c1, c2).astype(np.float32)
            return b, b.reshape(b.shape[0], -1).sum(axis=-1, keepdims=True)
        return _r

    defs = [
        ("EXPACC_SUM_ANT",
         Spec(body=Src0 + Src1, accum=add,
              reference=_ref_sum(lambda in0, in1, c0, c1, c2:
                                 in0.astype(np.float32) + in1))),
        ("EXPCPY_SUM_ANT",
         Spec(body=Src0 + Zero, accum=add,
              reference=_ref_sum(lambda in0, in1, c0, c1, c2:
                                 in0.astype(np.float32)))),
    ]
    for name, spec in defs:
        if name in _SUB_OPCODE_FOR_NAME:
            op = next(o for o in OPS if o.name == name)
            _DVE_OPS[name] = op
            continue
        row = max(_SUB_OPCODE_FOR_NAME.values()) + 1
        assert row < 0x20, "custom-DVE row field overflow"
        _SUB_OPCODE_FOR_NAME[name] = row
        shas = {}
        for ver in ("v3", "v4"):
            s = DveOpSpec(name=name, opcode=row, uops=lower(spec, ver=ver),
                          rd1_en=has_src1(spec))
            shas[ver] = s.sha(ver)
        op = DveOp(name, spec, subdim=False, uops_sha=shas)
        OPS.append(op)
        CUSTOM_DVE_SPECS[name] = spec
        _DVE_OPS[name] = op
    return _DVE_OPS


def build(b_total=B, d_img=D_IMG, d_num=D_NUM, n_cores=N_CORES):
    """Build + compile the Bass module. Returns the compiled Bacc object."""
    key = (b_total, d_img, d_num, n_cores)
    if key in _NC_CACHE:
        return _NC_CACHE[key]

    import concourse.bacc as bacc
    import concourse.mybir as mybir
    import concourse.tile as tile

    ops = _register_dve_ops()
    EXPACC = ops["EXPACC_SUM_ANT"]
    EXPCPY = ops["EXPCPY_SUM_ANT"]

    dt = mybir.dt
    AF = mybir.ActivationFunctionType
    Alu = mybir.AluOpType
    AX = mybir.AxisListType
    f32 = dt.float32
    bf16 = dt.bfloat16
    i16 = dt.int16

    BL = b_total // n_cores          # local rows per core
    assert BL % 512 == 0 and b_total % 1024 == 0
    NRT = BL // 512                  # 512-wide row tiles (MLP / transpose)
    NRC = BL // 128                  # 128-row chunks (main pass)
    KI = d_img // 128                # contraction tiles, img MLP1
    KN = d_num // 128
    CW = 1024                        # main-pass column supertile width
    NCT = b_total // CW
    NH = CW // 512
    ARW = b_total + 64               # AllReduce payload width

    # D-path (Vector-engine exp) row chunks, interleaved among ACT chunks.
    D_RC = [rc for rc in (2, 5, 8, 11, 13, 15) if rc < NRC]
    if not D_RC:
        D_RC = [NRC - 1]
    D_SET = set(D_RC)
    D_IDX = {rc: i for i, rc in enumerate(D_RC)}
    A_RC = [rc for rc in range(NRC) if rc not in D_SET]
    A_IDX = {rc: i for i, rc in enumerate(A_RC)}
    ND = len(D_RC)
    NA = len(A_RC)

    nc = bacc.Bacc("TRN2", target_bir_lowering=False, debug=False,
                   num_devices=n_cores)

    img = nc.dram_tensor("img_feat", [BL, d_img], f32, kind="ExternalInput").ap()
    num = nc.dram_tensor("num_feat", [BL, d_num], f32, kind="ExternalInput").ap()
    Wi1 = nc.dram_tensor("Wi1", [d_img, P], f32, kind="ExternalInput").ap()
    bi1 = nc.dram_tensor("bi1", [P, 1], f32, kind="ExternalInput").ap()
    Wi2 = nc.dram_tensor("Wi2", [P, P], f32, kind="ExternalInput").ap()
    bi2 = nc.dram_tensor("bi2", [P, 1], f32, kind="ExternalInput").ap()
    Wn1 = nc.dram_tensor("Wn1", [d_num, P], f32, kind="ExternalInput").ap()
    bn1 = nc.dram_tensor("bn1", [P, 1], f32, kind="ExternalInput").ap()
    Wn2 = nc.dram_tensor("Wn2", [P, P], f32, kind="ExternalInput").ap()
    bn2 = nc.dram_tensor("bn2", [P, 1], f32, kind="ExternalInput").ap()
    ltm = nc.dram_tensor("log_temp", [1, 1], f32, kind="ExternalInput").ap()
    loss = nc.dram_tensor("loss", [1, 1], f32, kind="ExternalOutput").ap()

    rg = [list(range(n_cores))]

    with tile.TileContext(nc) as tc:
        with (
            tc.tile_pool(name="sb", bufs=1) as sb,
            tc.tile_pool(name="stream", bufs=3) as st,
            tc.tile_pool(name="vstage", bufs=2) as vs,
            tc.tile_pool(name="dram", bufs=1, space="DRAM") as dram,
        ):
            xsp_pool = tc.tile_pool(name="xsp", bufs=2)
            xsp = xsp_pool.__enter__()
            xtp_pool = tc.tile_pool(name="xtp", bufs=2)
            xtp = xtp_pool.__enter__()
            # ---------------- early img DMA (biggest input: 8MB bf16) ------
            # Launched first on the gpsimd SWDGE queue so HBM is busy from
            # t=0; the AllGather triggers are interleaved after the first two
            # row-tiles so they don't stall behind tile-pool waits.
            xs_list = []
            for rb in range(min(2, NRT)):
                rsl = slice(rb * 512, (rb + 1) * 512)
                xs = xsp.tile([P, 4, d_img], bf16, tag="xsi", name="xsi")
                nc.gpsimd.dma_start(
                    xs[:], img[rsl, :].rearrange("(q p) e -> p q e", p=P))
                xs_list.append(xs)

            # num input on the sync queue -- it gates the AllGather chain.
            # (scoped pool: freed after the num branch to reclaim 16KB/part)
            nin_pool = tc.tile_pool(name="nin", bufs=1)
            nin = nin_pool.__enter__()
            xs_n = nin.tile([P, NRC, d_num], f32)
            nc.sync.dma_start(xs_n[:], num.rearrange("(g p) e -> p g e", p=P))

            # ---------------- constants ----------------
            ones_kb = sb.tile([P, 1], bf16)
            nc.vector.memset(ones_kb[:], 1.0)
            ones_kf = sb.tile([P, 1], f32)
            nc.vector.memset(ones_kf[:], 1.0)
            ones_1f = sb.tile([1, P], f32)
            nc.vector.memset(ones_1f[:], 1.0)
            zpad = sb.tile([1, 64], f32)
            nc.vector.memset(zpad[:], 0.0)
            # identity (bf16/f32) for PE-mode transposes
            idn_i = sb.tile([P, P], dt.int32)
            nc.gpsimd.iota(idn_i[:], pattern=[[1, P]], base=0,
                           channel_multiplier=-1)
            idn = sb.tile([P, P], bf16)
            nc.vector.tensor_scalar(idn[:], idn_i[:], 0, None,
                                    op0=Alu.is_equal)
            idn_f = sb.tile([P, P], f32)
            nc.vector.tensor_scalar(idn_f[:], idn_i[:], 0, None,
                                    op0=Alu.is_equal)

            # num-branch weights via HWDGE (sync) as fp32 + DVE cast.
            wn1_f = sb.tile([P, KN * P], f32)
            nc.sync.dma_start(wn1_f.rearrange("p (k m) -> p k m", k=KN),
                              Wn1.rearrange("(k p) m -> p k m", p=P))
            wn1_sb = sb.tile([P, KN * P], bf16)
            nc.vector.tensor_copy(wn1_sb[:], wn1_f[:])
            wn2_f = sb.tile([P, P], f32)
            nc.sync.dma_start(wn2_f[:], Wn2)
            wn2_sb = sb.tile([P, P], bf16)
            nc.vector.tensor_copy(wn2_sb[:], wn2_f[:])
            bn1_sb = sb.tile([P, 1], f32)
            nc.sync.dma_start(bn1_sb[:], bn1)
            bn2_sb = sb.tile([P, 1], f32)
            nc.sync.dma_start(bn2_sb[:], bn2)
            bi1_sb = sb.tile([P, 1], f32)
            nc.sync.dma_start(bi1_sb[:], bi1)
            bi2_sb = sb.tile([P, 1], f32)
            nc.sync.dma_start(bi2_sb[:], bi2)
            lt_sb = sb.tile([1, 1], f32)
            nc.sync.dma_start(lt_sb[:], ltm)
            nhlt = sb.tile([1, 1], f32)        # -0.5 * log_temp
            nc.vector.tensor_scalar_mul(nhlt[:], lt_sb[:], -0.5)

            # ---------------- DRAM scratch ----------------
            BH = BL // 2
            ag_in_a = dram.tile([P, BH], bf16)
            ag_in_b = dram.tile([P, BH], bf16)
            ag_out_a = dram.tile([n_cores * P, BH], bf16, addr_space="Shared")
            ag_out_b = dram.tile([n_cores * P, BH], bf16, addr_space="Shared")
            ARH = b_total // 2
            ar_in = dram.tile([1, ARW], f32)
            ar_out_a = dram.tile([1, ARH], f32, addr_space="Shared")
            ar_out_b = dram.tile([1, ARW - ARH], f32, addr_space="Shared")

            # ---------------- persistent SBUF ----------------
            xnT = sb.tile([P, KN * BL], bf16)   # num input, transposed
            h1n = sb.tile([P, BL], bf16)
            h1i = sb.tile([P, BL], bf16)
            zn = sb.tile([P, BL], bf16)
            zi = sb.tile([P, BL], bf16)
            ntl = sb.tile([P, BL], bf16)        # normalized num proj (local)
            itl = sb.tile([P, BL], bf16)        # normalized img proj (local)
            npf = sb.tile([P, b_total], bf16)   # gathered num proj (all cores)
            rowacc = sb.tile([P, NA * NCT], f32)  # ACT-row sums (accum_out)
            sacc = sb.tile([P, ND * NCT], f32)    # DVE-row telescoped sums
            dsum = sb.tile([1, 1], f32)         # running sum of diag
            nc.vector.memset(dsum[:], 0.0)

            vrow = sb.tile([1, BL], f32)

            def mlp2_norm(pp, h1, w2, b2, z, outp):
                """z = w2.T@h1 + b2; outp = z * inv with
                inv = exp(-0.5*ln(|z|^2) - 0.5*log_temp).  Bias-add and
                squaring run on DVE; only Ln/Exp (batched, one instr each)
                touch ACT so the activation table is not thrashed."""
                for rt in range(NRT):
                    sl = slice(rt * 512, (rt + 1) * 512)
                    pz = pp.tile([P, 512], f32, tag="zb", name="pz")
                    nc.tensor.matmul(pz[:], w2[:], h1[:, sl])
                    nc.vector.tensor_scalar(z[:, sl], pz[:], b2[:], None,
                                            op0=Alu.add)
                    sq = st.tile([P, 512], bf16, tag="sq", name="sq")
                    nc.vector.tensor_mul(sq[:], z[:, sl], z[:, sl])
                    pv = pp.tile([P, 512], f32, tag="v", name="pv")
                    nc.tensor.matmul(pv[:1, :], ones_kb[:], sq[:])
                    nc.vector.tensor_copy(vrow[:1, sl], pv[:1, :])
                lnv = vs.tile([1, BL], f32, tag="lnv", name="lnv", bufs=1)
                nc.scalar.activation(lnv[:], vrow[:1, :], AF.Ln)
                inv = vs.tile([1, BL], f32, tag="inv", name="inv", bufs=1)
                nc.scalar.activation(inv[:], lnv[:], AF.Exp,
                                     bias=nhlt[:], scale=-0.5)
                for rt in range(NRT):
                    sl = slice(rt * 512, (rt + 1) * 512)
                    pb = pp.tile([P, 512], f32, tag="zb", name="pb")
                    nc.tensor.matmul(pb[:], ones_1f[:], inv[:, sl])
                    nc.vector.tensor_mul(outp[:, sl], z[:, sl], pb[:])

            # ---------------- num branch + AllGather ----------------
            with tc.tile_pool(name="pp1", bufs=2, space="PSUM") as pp:
                for dk in range(KN):
                    for gb in range(NRC // 4):
                        pt = pp.tile([P, 512], f32, tag="pt", name="ptn")
                        for q in range(4):
                            nc.tensor.transpose(
                                pt[:, q * P:(q + 1) * P],
                                xs_n[:, gb * 4 + q, dk * P:(dk + 1) * P],
                                idn_f[:])
                        nc.vector.tensor_copy(
                            xnT[:, dk * BL + gb * 512: dk * BL + gb * 512 + 512],
                            pt[:])
                for rt in range(NRT):
                    sl = slice(rt * 512, (rt + 1) * 512)
                    ph = pp.tile([P, 512], f32, tag="h", name="ph")
                    for k in range(KN):
                        nc.tensor.matmul(
                            ph[:], wn1_sb[:, k * P:(k + 1) * P],
                            xnT[:, k * BL + rt * 512: k * BL + rt * 512 + 512],
                            start=(k == 0), stop=(k == KN - 1))
                    nc.scalar.activation(h1n[:, sl], ph[:], AF.Relu, bias=bn1_sb[:])
                mlp2_norm(pp, h1n, wn2_sb, bn2_sb, zn, ntl)
            nin_pool.__exit__(None, None, None)

            npf_v = npf.rearrange("p (r c) -> p r c", c=BL)
            nc.sync.dma_start(ag_in_a[:], ntl[:, 0:BH])
            nc.gpsimd.collective_compute(
                "AllGather", Alu.bypass, replica_groups=rg,
                ins=[ag_in_a.opt()], outs=[ag_out_a.opt()])
            nc.sync.dma_start(ag_in_b[:], ntl[:, BH:BL])
            nc.gpsimd.collective_compute(
                "AllGather", Alu.bypass, replica_groups=rg,
                ins=[ag_in_b.opt()], outs=[ag_out_b.opt()])
            nc.sync.dma_start(npf_v[:, :, 0:BH],
                              ag_out_a.rearrange("(r p) n -> p r n", p=P))
            nc.sync.dma_start(npf_v[:, :, BH:BL],
                              ag_out_b.rearrange("(r p) n -> p r n", p=P))

            # remaining img tiles + img weights (SWDGE queue, after AG trigger)
            for rb in range(min(2, NRT), NRT):
                rsl = slice(rb * 512, (rb + 1) * 512)
                xs = xsp.tile([P, 4, d_img], bf16, tag="xsi", name="xsi")
                nc.gpsimd.dma_start(
                    xs[:], img[rsl, :].rearrange("(q p) e -> p q e", p=P))
                xs_list.append(xs)
            wi1_sb = sb.tile([P, KI * P], bf16)
            nc.gpsimd.dma_start(wi1_sb.rearrange("p (k m) -> p k m", k=KI),
                                Wi1.rearrange("(k p) m -> p k m", p=P))
            wi2_sb = sb.tile([P, P], bf16)
            nc.gpsimd.dma_start(wi2_sb[:], Wi2)

            # ---------------- img branch ----------------
            with tc.tile_pool(name="pp2", bufs=2, space="PSUM") as pp:
                for rb in range(NRT):
                    rsl = slice(rb * 512, (rb + 1) * 512)
                    xs = xs_list[rb]
                    xtb = xtp.tile([P, KI * 512], bf16, tag="xt", name="xtb")
                    for dk in range(KI):
                        pt = pp.tile([P, 512], bf16, tag="pt", name="pt")
                        for q in range(4):
                            nc.tensor.transpose(
                                pt[:, q * P:(q + 1) * P],
                                xs[:, q, dk * P:(dk + 1) * P], idn[:])
                        nc.vector.tensor_copy(
                            xtb[:, dk * 512:(dk + 1) * 512], pt[:])
                    ph = pp.tile([P, 512], f32, tag="h", name="phi")
                    for k in range(KI):
                        nc.tensor.matmul(
                            ph[:], wi1_sb[:, k * P:(k + 1) * P],
                            xtb[:, k * 512:(k + 1) * 512],
                            start=(k == 0), stop=(k == KI - 1))
                    nc.scalar.activation(h1i[:, rsl], ph[:], AF.Relu,
                                         bias=bi1_sb[:])
                mlp2_norm(pp, h1i, wi2_sb, bi2_sb, zi, itl)
                # diagonal: l_ii = sum_p itl[p,i] * ntl[p,i]; accumulate sum
                for rt in range(NRT):
                    sl = slice(rt * 512, (rt + 1) * 512)
                    prod = st.tile([P, 512], bf16, tag="sq", name="prod")
                    nc.vector.tensor_mul(prod[:], itl[:, sl], ntl[:, sl])
                    pd = pp.tile([P, 512], f32, tag="h", name="pd")
                    nc.tensor.matmul(pd[:1, :], ones_kb[:], prod[:])
                    dred = vs.tile([1, 1], f32, tag="dred", name="dred")
                    nc.vector.reduce_sum(dred[:], pd[:1, :], axis=AX.X)
                    nc.vector.tensor_add(dsum[:], dsum[:], dred[:])
            xtp_pool.__exit__(None, None, None)
            xsp_pool.__exit__(None, None, None)

            # ---------------- main pass ----------------
            # ct order: supertiles covered by AG half a first.
            ct_a = [ct for ct in range(NCT)
                    if (ct * CW) % BL + CW <= BH]
            ct_order = ct_a + [ct for ct in range(NCT) if ct not in ct_a]
            NHALF = NCT // 2
            with (
                tc.tile_pool(name="pl", bufs=3, space="PSUM") as plp,
                tc.tile_pool(name="pc", bufs=1, space="PSUM") as pcp,
                tc.tile_pool(name="epool", bufs=4) as ep,
                tc.tile_pool(name="accp", bufs=2) as ap_,
                tc.tile_pool(name="eip", bufs=2) as eip,
            ):
                for pos, ct in enumerate(ct_order):
                    pcol = pcp.tile([P, CW], f32, tag="pc", name="pcol")
                    acc_prev = None
                    first_col = True
                    for rc in range(NRC):
                        plog = plp.tile([P, CW], f32, tag="pl", name="plog")
                        for h in range(NH):
                            nc.tensor.matmul(
                                plog[:, h * 512:(h + 1) * 512],
                                itl[:, rc * P:(rc + 1) * P],
                                npf[:, ct * CW + h * 512: ct * CW + (h + 1) * 512])
                        if rc not in D_SET:
                            # ACT path: exp + fused row accumulation
                            e = ep.tile([P, CW], bf16, tag="e", name="e")
                            slot = A_IDX[rc] * NCT + ct
                            nc.scalar.activation(
                                e[:], plog[:], AF.Exp,
                                accum_out=rowacc[:, slot:slot + 1])
                            for h in range(NH):
                                nc.tensor.matmul(
                                    pcol[:1, h * 512:(h + 1) * 512],
                                    ones_kb[:], e[:, h * 512:(h + 1) * 512],
                                    start=first_col, stop=False)
                            first_col = False
                        else:
                            # DVE path: Schraudolph exp (int16 bits = bf16 e)
                            ei = eip.tile([P, CW], i16, tag="ei", name="ei")
                            nc.vector.tensor_scalar(
                                ei[:], plog[:], SCH_A, SCH_B,
                                op0=Alu.mult, op1=Alu.add)
                            ev = ei[:].bitcast(bf16)
                            slot = D_IDX[rc] * NCT + ct
                            acc = ap_.tile([P, CW], bf16, tag="acc", name="acc")
                            if acc_prev is None:
                                nc.vector._custom_dve(
                                    EXPCPY, out=acc[:], in0=ev,
                                    accum_out=sacc[:, slot:slot + 1])
                            else:
                                nc.vector._custom_dve(
                                    EXPACC, out=acc[:], in0=ev, in1=acc_prev[:],
                                    accum_out=sacc[:, slot:slot + 1])
                            acc_prev = acc
                    # fold the DVE-row column accumulator into pcol, close
                    # the PSUM accumulation group, ship colsums to ar_in.
                    for h in range(NH):
                        nc.tensor.matmul(
                            pcol[:1, h * 512:(h + 1) * 512],
                            ones_kb[:], acc_prev[:, h * 512:(h + 1) * 512],
                            start=False, stop=True)
                    cst = vs.tile([1, CW], f32, tag="cst", name="cst")
                    if pos % 2 == 0:
                        nc.scalar.activation(cst[:], pcol[:1, :], AF.Copy)
                    else:
                        nc.vector.tensor_copy(cst[:], pcol[:1, :])
                    nc.sync.dma_start(ar_in[:1, pos * CW:(pos + 1) * CW], cst[:])
                    if pos == NHALF - 1:
                        nc.gpsimd.collective_compute(
                            "AllReduce", Alu.add, replica_groups=rg,
                            ins=[ar_in[:1, 0:ARH].opt()], outs=[ar_out_a.opt()])

                # ---- row direction partials ----
                rs_all = sb.tile([P, NRC], f32)
                nc.vector.reduce_sum(
                    rs_all[:, 0:NA],
                    rowacc.rearrange("p (a ct) -> p a ct", ct=NCT), axis=AX.X)
                # telescoped DVE-row sums: S[d] - S[d-1] summed over ct
                dsd = sb.tile([P, ND * NCT], f32)
                nc.vector.tensor_copy(dsd[:, 0:NCT], sacc[:, 0:NCT])
                if ND > 1:
                    nc.vector.tensor_sub(dsd[:, NCT:], sacc[:, NCT:],
                                         sacc[:, 0:(ND - 1) * NCT])
                nc.vector.reduce_sum(
                    rs_all[:, NA:NRC],
                    dsd.rearrange("p (d ct) -> p d ct", ct=NCT), axis=AX.X)
                lse_r = sb.tile([P, NRC], f32)
                nc.scalar.activation(lse_r[:], rs_all[:], AF.Ln)
                lsum = sb.tile([P, 1], f32)
                nc.vector.reduce_sum(lsum[:], lse_r[:], axis=AX.X)
                pR = pcp.tile([P, CW], f32, tag="pc", name="pR")
                nc.tensor.matmul(pR[:1, :1], ones_kf[:], lsum[:])
                rpart = sb.tile([1, 1], f32)
                nc.vector.tensor_sub(rpart[:], pR[:1, :1], dsum[:])
                nc.sync.dma_start(ar_in[:1, b_total:b_total + 1], rpart[:])
                nc.sync.dma_start(ar_in[:1, b_total + 1:b_total + 2], dsum[:])
                nc.sync.dma_start(ar_in[:1, b_total + 2:ARW], zpad[:1, :62])

                # ---- AllReduce (second half + scalars) ----
                nc.gpsimd.collective_compute(
                    "AllReduce", Alu.add, replica_groups=rg,
                    ins=[ar_in[:1, ARH:ARW].opt()], outs=[ar_out_b.opt()])

                # ---- final ----
                HB2 = b_total - ARH
                csb = sb.tile([P, b_total // P], f32)
                nc.sync.dma_start(
                    csb[:, :ARH // P],
                    ar_out_a.rearrange("o (a b) -> (o a) b", a=P))
                nc.sync.dma_start(
                    csb[:, ARH // P:],
                    ar_out_b[:1, :HB2].rearrange("o (a b) -> (o a) b", a=P))
                sc2 = sb.tile([1, 2], f32)
                nc.sync.dma_start(sc2[:], ar_out_b[:1, HB2:HB2 + 2])
                lse_c = sb.tile([P, b_total // P], f32)
                nc.scalar.activation(lse_c[:], csb[:], AF.Ln)
                csum_p = sb.tile([P, 1], f32)
                nc.vector.reduce_sum(csum_p[:], lse_c[:], axis=AX.X)
                pC = pcp.tile([P, CW], f32, tag="pc", name="pC")
                nc.tensor.matmul(pC[:1, :1], ones_kf[:], csum_p[:])
                t1 = sb.tile([1, 1], f32)
                nc.vector.tensor_add(t1[:], pC[:1, :1], sc2[:1, 0:1])
                t2 = sb.tile([1, 1], f32)
                nc.vector.tensor_sub(t2[:], t1[:], sc2[:1, 1:2])
                lsb = sb.tile([1, 1], f32)
                nc.vector.tensor_scalar_mul(lsb[:], t2[:], 1.0 / (2.0 * b_total))
                nc.sync.dma_start(loss, lsb[:])

    nc.compile()
    _NC_CACHE[key] = nc
    return nc


def shard_inputs(inputs, b_total=B, n_cores=N_CORES):
    BL = b_total // n_cores
    img = np.ascontiguousarray(np.asarray(inputs["img_feat"], dtype=np.float32))
    num = np.ascontiguousarray(np.asarray(inputs["num_feat"], dtype=np.float32))

    def mat(name):
        return np.ascontiguousarray(np.asarray(inputs[name], dtype=np.float32))

    def col(name):
        return np.ascontiguousarray(
            np.asarray(inputs[name], dtype=np.float32).reshape(P, 1))

    lt = np.asarray(inputs["log_temp"], dtype=np.float32).reshape(1, 1)
    shared = {
        "Wi1": mat("Wi1"), "Wi2": mat("Wi2"),
        "Wn1": mat("Wn1"), "Wn2": mat("Wn2"),
        "bi1": col("bi1"), "bi2": col("bi2"),
        "bn1": col("bn1"), "bn2": col("bn2"),
        "log_temp": np.ascontiguousarray(lt),
    }
    maps = []
    for c in range(n_cores):
        m = dict(shared)
        m["img_feat"] = np.ascontiguousarray(img[c * BL:(c + 1) * BL])
        m["num_feat"] = np.ascontiguousarray(num[c * BL:(c + 1) * BL])
        maps.append(m)
    return maps


def run(inputs, trace=False, **kw):
    """Run on hardware; returns (loss_scalar, BassKernelResults)."""
    from concourse.bass_utils import run_bass_kernel_spmd
    nc = build()
    res = run_bass_kernel_spmd(nc, shard_inputs(inputs),
                               core_ids=list(range(N_CORES)), trace=trace, **kw)
    val = np.asarray(res.results[0]["loss"], dtype=np.float32).reshape(())
    return val, res


def kernel(**inputs):
    val, _ = run(inputs)
    return val
